# revision 28
# baseline (speedup 1.0000x reference)
"""AntiSymmetricConv (GNN message passing) on 8 TRN2 NeuronCores — v2.

Strategy (dst-sharded pull mode, dma_gather based):
  - Host: degree-sort dst nodes, assign 128-node tiles round-robin to the 8
    cores (load balance + one SPMD graph).  Node (core c, tile t, slot p)
    owns y-row R = c*12800 + p*100 + t of the AllGathered y_full.  y_full is
    bf16 and split into 4 windows of 25600 rows (= 2 shards) so that
    window-relative row indices fit dma_gather's int16 index format.
  - Per (dst tile i, src window w), in-edges are packed into chunks of 128;
    the chunk schedule K(i,w) = max over cores (SPMD).  Each edge also
    carries its dst slot (for on-device one-hot scatter matrices).
  - Device, per iteration:
      phase A per tile: transpose x (PE), [xw|xa] = x @ [W_phi.T|A.T] (bf16
        matmul), y = xw*dinv_y -> y_sb (bf16), xa += bias -> xa_sb.
      One flat DMA y_sb -> y_bounce; AllGather -> y_full [102400,128] bf16.
      phase C per group of tiles (packed to <=GCH chunks): one dma_gather
        per (group, window) pulls all chunks' src rows (128 rows/chunk, one
        SWDGE instruction for thousands of descriptors); per tile: one
        batched DVE is_equal generates all one-hot scatter matrices S;
        K matmuls accumulate S^T @ rows into PSUM + identity matmul adds
        the self-loop; fused epilogue h = tanh(agg*dinv + xa), x += 0.1*h
        (x kept in DRAM).
  - Output: read back x shards, invert the permutation on host.
"""

import os

import numpy as np
import ml_dtypes

import concourse.bacc as bacc
import concourse.bass as bass
import concourse.mybir as mybir
import concourse.tile as tile
from concourse.bass_utils import run_bass_kernel_spmd
from concourse.masks import make_identity

GAMMA = 0.1
EPSILON = 0.1
NUM_ITERS = 4
P = 128   # partitions / tile size
C = 8     # cores
D = 128   # feature dim
NT = 100  # tiles per core
NPC = NT * P          # rows per core (12800)
NWIN = 4              # gather windows (2 shards each)
WROWS = 2 * NPC       # rows per window (25600)
ZERO_T = 98           # all-pad tile index on every core
GCH = 96              # max chunks per phase-C group (SBUF budget)
SLAB = 64             # max chunks per dma_gather (8192 idxs, multi-packet)
USE_FP8 = True        # fp8e4 DoubleRow matmuls: 2 chunks per PE instruction

FP = mybir.dt.float32
BF = mybir.dt.bfloat16
F8 = mybir.dt.float8e4
I16 = mybir.dt.int16

BF_NP = ml_dtypes.bfloat16


def _pack_idxs(idx_flat: np.ndarray) -> np.ndarray:
    """[n] -> [128, n//16] int16: idx j at (partition j%16, col j//16),
    replicated x8 across partition groups of 16."""
    n = idx_flat.shape[0]
    assert n % 16 == 0
    t = idx_flat.reshape(n // 16, 16).T.astype(np.int16)
    return np.tile(t, (8, 1))


# ----------------------------------------------------------------- host prep
def _preprocess(x, edge_index, W, W_phi, bias):
    N = x.shape[0]
    src = edge_index[0].astype(np.int64)
    dst = edge_index[1].astype(np.int64)

    deg = np.bincount(dst, minlength=N).astype(np.float64) + 1.0
    dinv = (1.0 / np.sqrt(deg)).astype(np.float32)

    # global degree-descending order; rank r -> (g=r//P, slot=r%P),
    # core = g%C, tile = g//C
    order = np.argsort(-deg, kind="stable")
    rank = np.empty(N, dtype=np.int64)
    rank[order] = np.arange(N)

    g_of = rank // P
    core_of = (g_of % C).astype(np.int64)
    tile_of = (g_of // C).astype(np.int64)
    slot_of = (rank % P).astype(np.int64)
    # tiles 98,99 are all-pad on every core for N=100000 (800 global tiles)
    assert tile_of.max() <= ZERO_T - 1, tile_of.max()

    # y_full row (global) and window-relative index
    yrow = core_of * NPC + slot_of * NT + tile_of
    win_of = core_of // 2
    widx = yrow - win_of * WROWS  # in [0, WROWS)

    # ---- per-core node data (x upload: tile-major rows r = t*128 + p)
    node_at = np.full((C, NT, P), -1, dtype=np.int64)
    node_at[core_of, tile_of, slot_of] = np.arange(N)
    valid = node_at >= 0
    nid = np.where(valid, node_at, 0)
    x_mem = x[nid.reshape(C, -1)].astype(np.float32)   # [C, NT*P, D]
    x_mem[~valid.reshape(C, -1)] = 0.0

    dv = dinv[nid]                                     # [C, NT, P]
    dinv_sb = np.where(valid, dv, 1.0).astype(np.float32)
    dinv_y_sb = np.where(valid, dv, 0.0).astype(np.float32)
    dinv_sb = np.ascontiguousarray(dinv_sb.transpose(0, 2, 1))      # [C,P,NT]
    dinv_y_sb = np.ascontiguousarray(dinv_y_sb.transpose(0, 2, 1))

    # ---- edge schedule
    e_core = core_of[dst]
    e_tile = tile_of[dst]
    e_slot = slot_of[dst]
    e_win = win_of[src]
    e_widx = widx[src]

    key = (e_core * NT + e_tile) * NWIN + e_win
    cnt = np.bincount(key, minlength=C * NT * NWIN).reshape(C, NT, NWIN)
    K_need = -(-cnt // P)
    K_sched = K_need.max(axis=0)               # [NT, NWIN] shared schedule
    CHT = int(K_sched.sum())
    Ktot_i = K_sched.sum(axis=1)               # [NT]
    KMAX = int(Ktot_i.max())

    # order edges by (core, tile, window), stable; position within group
    eorder = np.argsort(key, kind="stable")
    key_s = key[eorder]
    first = np.searchsorted(key_s, key_s)
    j_in_grp = np.arange(len(key_s)) - first

    # tile-major chunk id space: chunk (i, w, k) -> off_iw[i*NWIN+w] + k
    off_iw = np.zeros(NT * NWIN + 1, dtype=np.int64)
    off_iw[1:] = np.cumsum(K_sched.reshape(-1))
    e_i = (key_s // NWIN) % NT
    e_w = key_s % NWIN
    e_c = key_s // (NT * NWIN)
    chunk_of = off_iw[e_i * NWIN + e_w] + j_in_grp // P
    pos_of = j_in_grp % P

    widx_arr = np.full((C, P, CHT), ZERO_T, dtype=np.int32)  # zero row = 98
    sdst_arr = np.zeros((C, P, CHT), dtype=np.float32)
    widx_arr[e_c, pos_of, chunk_of] = e_widx[eorder].astype(np.int32)
    sdst_arr[e_c, pos_of, chunk_of] = e_slot[eorder].astype(np.float32)
    assert widx_arr.max() < WROWS

    # ---- variable groups: pack tiles while sum(Ktot) <= GCH
    groups = []  # list of tile-lists
    cur, cur_k = [], 0
    for i in range(NT):
        k = int(Ktot_i[i])
        if cur and cur_k + k > GCH:
            groups.append(cur)
            cur, cur_k = [], 0
        cur.append(i)
        cur_k += k
    if cur:
        groups.append(cur)
    assert max(int(Ktot_i[i]) for i in range(NT)) <= GCH

    # gather order: for group, for w, for tile in group, chunks k
    gorder = np.empty(CHT, dtype=np.int64)   # tile-major chunk -> gather pos
    gw_chunkoff = []                         # per group: [NWIN+1] offsets
    pos = 0
    for tiles in groups:
        offs = [pos]
        for w in range(NWIN):
            for i in tiles:
                k = int(K_sched[i, w])
                cbase = int(off_iw[i * NWIN + w])
                for kk in range(k):
                    gorder[cbase + kk] = pos
                    pos += 1
            offs.append(pos)
        gw_chunkoff.append(offs)
    assert pos == CHT
    inv_gorder = np.empty(CHT, dtype=np.int64)
    inv_gorder[gorder] = np.arange(CHT)
    widx_g = widx_arr[:, :, inv_gorder]      # chunks in gather order

    idx_packed = np.empty((C, P, 8 * CHT), dtype=np.int16)
    for c in range(C):
        flat = widx_g[c].T.reshape(-1)       # j = chunk*128 + p
        idx_packed[c] = _pack_idxs(flat)

    # matmul rhs [128, 256] bf16 = [W_phi.T | A.T]
    A = W - W.T - GAMMA * np.eye(D, dtype=np.float32)
    rhs = np.concatenate([W_phi.T, A.T], axis=1).astype(BF_NP)
    bias_b = np.tile(bias[None, :], (P, 1)).astype(BF_NP)

    in_maps = []
    for c in range(C):
        in_maps.append(
            {
                "x_in": np.ascontiguousarray(x_mem[c]),
                "dinv": np.ascontiguousarray(dinv_sb[c]),
                "dinv_y": np.ascontiguousarray(dinv_y_sb[c]),
                "idx_pk": np.ascontiguousarray(idx_packed[c]),
                "sdst": np.ascontiguousarray(sdst_arr[c].astype(BF_NP)),
                "rhs": rhs,
                "bias_b": bias_b,
            }
        )
    meta = dict(
        K_sched=K_sched, CHT=CHT, KMAX=KMAX, groups=groups,
        gw_chunkoff=gw_chunkoff, off_iw=off_iw, gorder=gorder,
        node_at=node_at, valid=valid, N=N,
    )
    return in_maps, meta


def _postprocess(results, meta):
    N = meta["N"]
    node_at, valid = meta["node_at"], meta["valid"]
    out = np.empty((N, D), dtype=np.float32)
    for c in range(C):
        xc = results[c]["x_out"].reshape(NT, P, D)
        v = valid[c]
        out[node_at[c][v]] = xc[v]
    return out


# ------------------------------------------------------------- device graph
def _build_graph(meta, n_iters=NUM_ITERS, debug_mode=None):
    K_sched = meta["K_sched"]
    CHT = int(meta["CHT"])
    KMAX = max(int(meta["KMAX"]), 1)
    groups = meta["groups"]
    gw_chunkoff = meta["gw_chunkoff"]
    off_iw = meta["off_iw"]
    gorder = meta["gorder"]

    nc = bacc.Bacc("TRN2", target_bir_lowering=False, debug=False,
                   num_devices=C)
    x_in = nc.declare_dram_parameter("x_in", [NPC, D], FP, isOutput=False)
    dinv_in = nc.declare_dram_parameter("dinv", [P, NT], FP, isOutput=False)
    dinvy_in = nc.declare_dram_parameter("dinv_y", [P, NT], FP,
                                         isOutput=False)
    idx_in = nc.declare_dram_parameter("idx_pk", [P, 8 * CHT], I16,
                                       isOutput=False)
    sdst_in = nc.declare_dram_parameter("sdst", [P, CHT], BF, isOutput=False)
    rhs_in = nc.declare_dram_parameter("rhs", [P, 2 * D], BF, isOutput=False)
    bias_in = nc.declare_dram_parameter("bias_b", [P, D], BF, isOutput=False)
    x_out = nc.declare_dram_parameter("x_out", [NPC, D], FP, isOutput=True)

    x_cur = nc.dram_tensor("x_cur", [NPC, D], FP)
    y_bounce = nc.dram_tensor("y_bounce", [NPC, D], BF)
    y_full = nc.dram_tensor("y_full", [C * NPC, D], BF, addr_space="Shared")

    with tile.TileContext(nc) as tc:
        with (
            tc.tile_pool(name="stat", bufs=1) as stat,
            tc.tile_pool(name="xio", bufs=3) as xio,
            tc.tile_pool(name="xt", bufs=2) as xtp,
            tc.tile_pool(name="gat", bufs=2) as gat,
            tc.tile_pool(name="g8p", bufs=2) as g8p,
            tc.tile_pool(name="idxp", bufs=2) as idxp,
            tc.tile_pool(name="sgen", bufs=2) as sgen,
            tc.tile_pool(name="ps", bufs=2, space="PSUM") as psp,
            tc.tile_pool(name="psagg", bufs=2, space="PSUM") as psagg,
        ):
            # ---- static data
            ident_f = stat.tile([P, P], FP)
            make_identity(nc, ident_f[:])
            ident_b = stat.tile([P, P], BF)
            nc.vector.tensor_copy(out=ident_b[:], in_=ident_f[:])
            iota_rep = stat.tile([P, KMAX * D], BF)
            nc.gpsimd.iota(
                iota_rep[:], pattern=[[0, KMAX], [1, D]], base=0,
                channel_multiplier=0, allow_small_or_imprecise_dtypes=True,
            )
            rhs_sb = stat.tile([P, 2 * D], BF)
            nc.sync.dma_start(rhs_sb[:], rhs_in[:])
            bias_sb = stat.tile([P, D], BF)
            nc.sync.dma_start(bias_sb[:], bias_in[:])
            dinv_sb = stat.tile([P, NT], FP)
            nc.sync.dma_start(dinv_sb[:], dinv_in[:])
            dinvy_sb = stat.tile([P, NT], FP)
            nc.sync.dma_start(dinvy_sb[:], dinvy_in[:])
            sdst_sb = stat.tile([P, CHT], BF)
            nc.sync.dma_start(sdst_sb[:], sdst_in[:])
            y_sb = stat.tile([P, NT * D], BF)
            xa_sb = stat.tile([P, NT * D], BF)



            def phase_a(it):
                for i in range(NT):
                    x_t = xio.tile([P, D], FP, tag="xa_in")
                    src_t = x_in if it == 0 else x_cur
                    nc.sync.dma_start(x_t[:], src_t[i * P:(i + 1) * P, :])
                    ps_t = psp.tile([P, D], FP, tag="ps_t", space="PSUM")
                    nc.tensor.transpose(out=ps_t[:], in_=x_t[:],
                                        identity=ident_f[:])
                    xT = xtp.tile([P, D], BF, tag="xT")
                    nc.vector.tensor_copy(out=xT[:], in_=ps_t[:])
                    ps_a = psp.tile([P, 2 * D], FP, tag="ps_a", space="PSUM")
                    nc.tensor.matmul(out=ps_a[:], lhsT=xT[:], rhs=rhs_sb[:],
                                     start=True, stop=True)
                    nc.vector.tensor_scalar(
                        out=y_sb[:, i * D:(i + 1) * D], in0=ps_a[:, 0:D],
                        scalar1=dinvy_sb[:, i:i + 1], scalar2=None,
                        op0=mybir.AluOpType.mult)
                    nc.vector.tensor_tensor(
                        out=xa_sb[:, i * D:(i + 1) * D], in0=ps_a[:, D:2 * D],
                        in1=bias_sb[:], op=mybir.AluOpType.add)
                # flat y copy: y_sb [p, (t d)] -> y_bounce rows p*NT+t
                nc.sync.dma_start(
                    out=y_bounce[:].rearrange("(p t) d -> p (t d)", p=P),
                    in_=y_sb[:])
                if debug_mode == "no_collective":
                    nc.sync.dma_start(out=y_full[0:NPC, :], in_=y_bounce[:])
                else:
                    nc.gpsimd.collective_compute(
                        "AllGather",
                        mybir.AluOpType.bypass,
                        replica_groups=[list(range(C))],
                        ins=[y_bounce[:].opt()],
                        outs=[y_full[:].opt()],
                    )

            def phase_c(it):
                for gi, tiles in enumerate(groups):
                    offs = gw_chunkoff[gi]
                    gbase, gend = int(offs[0]), int(offs[NWIN])
                    nchunks = gend - gbase
                    if nchunks == 0:
                        continue
                    if debug_mode not in ("no_gather", "abl_min"):
                        gbuf = gat.tile([P, GCH, D], BF, tag="gb")
                        idx_t = idxp.tile([P, 8 * GCH], I16, tag="idx")
                        nc.sync.dma_start(
                            idx_t[:, 0:8 * nchunks],
                            idx_in[:, 8 * gbase:8 * gend])
                    for w in range(NWIN):
                        c0, c1 = int(offs[w]), int(offs[w + 1])
                        if c1 == c0 or debug_mode in ("no_gather", "abl_min"):
                            continue
                        # HW caps: 1024 idxs single-packet, 8192 multi-packet
                        for s0 in range(c0, c1, SLAB):
                            s1 = min(s0 + SLAB, c1)
                            ns = s1 - s0
                            nc.gpsimd.dma_gather(
                                out_ap=gbuf[:, s0 - gbase:s1 - gbase, :],
                                in_ap=y_full[w * WROWS:(w + 1) * WROWS, :],
                                idxs_ap=idx_t[:, 8 * (s0 - gbase):
                                              8 * (s1 - gbase)],
                                num_idxs=ns * P,
                                num_idxs_reg=ns * P,
                                elem_size=D,
                                single_packet=(ns * P <= 1024),
                            )
                    if USE_FP8 and debug_mode not in ("no_gather", "abl_min"):
                        g8 = g8p.tile([P, GCH, D], F8, tag="g8")
                        nc.vector.tensor_copy(
                            out=g8[:, 0:nchunks, :],
                            in_=gbuf[:, 0:nchunks, :])
                        mm_src = g8
                        s_dt = F8
                    else:
                        mm_src = None
                        s_dt = BF
                    for i in tiles:
                        kt = int(K_sched[i].sum())
                        ps_g = psagg.tile([P, D], FP, tag="agg",
                                          space="PSUM")
                        if kt > 0 and debug_mode not in ("no_gather", "abl_min"):
                            src_buf = mm_src if USE_FP8 else gbuf
                            s_t = sgen.tile([P, KMAX, D], s_dt, tag="s_t")
                            s_off = int(off_iw[i * NWIN])
                            nc.vector.tensor_tensor(
                                out=s_t[:, 0:kt, :],
                                in0=iota_rep[:, 0:kt * D].rearrange(
                                    "p (k d) -> p k d", k=kt),
                                in1=sdst_sb[:, s_off:s_off + kt]
                                .broadcast_to([P, kt, D]),
                                op=mybir.AluOpType.is_equal)
                            # gbuf columns of this tile's chunks, w-major
                            # (strictly increasing, matching s_t column m)
                            cols = []
                            for w in range(NWIN):
                                kw = int(K_sched[i, w])
                                if kw == 0:
                                    continue
                                cc = int(gorder[off_iw[i * NWIN + w]]) - gbase
                                cols.extend(range(cc, cc + kw))
                            m = 0
                            while m < kt:
                                if USE_FP8 and m + 1 < kt:
                                    c1, c2 = cols[m], cols[m + 1]
                                    nc.tensor.matmul(
                                        out=ps_g[:],
                                        lhsT=s_t[:, m:m + 2, :],
                                        rhs=src_buf[:, c1:c2 + 1:c2 - c1, :],
                                        start=(m == 0), stop=False,
                                        perf_mode=mybir.MatmulPerfMode
                                        .DoubleRow)
                                    m += 2
                                else:
                                    nc.tensor.matmul(
                                        out=ps_g[:],
                                        lhsT=s_t[:, m, :],
                                        rhs=src_buf[:, cols[m], :],
                                        start=(m == 0), stop=False)
                                    m += 1
                        # self-loop
                        nc.tensor.matmul(
                            out=ps_g[:], lhsT=ident_b[:],
                            rhs=y_sb[:, i * D:(i + 1) * D],
                            start=(kt == 0
                                   or debug_mode in ("no_gather", "abl_min")),
                            stop=True)
                        # t4 = agg*dinv + xa, overwriting the xa slice
                        nc.vector.scalar_tensor_tensor(
                            out=xa_sb[:, i * D:(i + 1) * D], in0=ps_g[:],
                            scalar=dinv_sb[:, i:i + 1],
                            in1=xa_sb[:, i * D:(i + 1) * D],
                            op0=mybir.AluOpType.mult,
                            op1=mybir.AluOpType.add)
                # batched epilogue: h = tanh(t4) in place, then quartered
                # x update x += eps*h (x stays in DRAM, staged per quarter)
                nc.scalar.activation(
                    out=xa_sb[:], in_=xa_sb[:],
                    func=mybir.ActivationFunctionType.Tanh)
                QT = NT // 4
                for q in range(4):
                    r0, r1 = q * QT * P, (q + 1) * QT * P
                    xq = xio.tile([P, QT * D], FP, tag="xq", bufs=2)
                    src_t = x_in if it == 0 else x_cur
                    nc.sync.dma_start(
                        xq[:].rearrange("p (t d) -> p t d", t=QT),
                        src_t[r0:r1, :].rearrange("(t p) d -> p t d", p=P))
                    nc.vector.scalar_tensor_tensor(
                        out=xq[:], in0=xa_sb[:, q * QT * D:(q + 1) * QT * D],
                        scalar=EPSILON, in1=xq[:],
                        op0=mybir.AluOpType.mult, op1=mybir.AluOpType.add)
                    dst_t = x_out if it == n_iters - 1 else x_cur
                    nc.sync.dma_start(
                        dst_t[r0:r1, :].rearrange("(t p) d -> p t d", p=P),
                        xq[:].rearrange("p (t d) -> p t d", t=QT))

            if n_iters == 0:
                nc.sync.dma_start(out=x_out[:], in_=x_in[:])
                # Fixed per-execution overhead parity with the full graph:
                # the SWDGE gather ucode library load, collective init, and
                # act table load are per-NEFF-execution constants, so the
                # overhead-only graph must pay them too for t_full - t_zero
                # to isolate the marginal compute.
                zscr = nc.dram_tensor("zscr", [P, D], FP)
                zidx = stat.tile([P, 8], I16)
                nc.vector.memset(zidx[:], 0)
                zg = stat.tile([P, 1, D], BF)
                nc.gpsimd.dma_gather(
                    out_ap=zg[:], in_ap=y_full[0:WROWS, :],
                    idxs_ap=zidx[:], num_idxs=P, num_idxs_reg=P,
                    elem_size=D)
                zt = stat.tile([P, D], FP)
                nc.scalar.activation(
                    out=zt[:], in_=zg[:, 0, :],
                    func=mybir.ActivationFunctionType.Tanh)
                nc.sync.dma_start(zscr[:], zt[:])
                nc.gpsimd.collective_compute(
                    "AllGather",
                    mybir.AluOpType.bypass,
                    replica_groups=[list(range(C))],
                    ins=[y_bounce[:].opt()],
                    outs=[y_full[:].opt()],
                )
            for it in range(n_iters):
                phase_a(it)
                phase_c(it)
    nc.compile()
    return nc


# ------------------------------------------------------------------- driver
_LAST = {}


def kernel(x, edge_index, W, W_phi, bias):
    x = np.asarray(x, dtype=np.float32)
    edge_index = np.asarray(edge_index, dtype=np.int32)
    W = np.asarray(W, dtype=np.float32)
    W_phi = np.asarray(W_phi, dtype=np.float32)
    bias = np.asarray(bias, dtype=np.float32)

    in_maps, meta = _preprocess(x, edge_index, W, W_phi, bias)
    nc = _build_graph(meta)
    trace = os.environ.get("BASS_PROFILE", "0") == "1"
    res = run_bass_kernel_spmd(
        nc, in_maps, core_ids=list(range(C)), trace=trace
    )
    _LAST["res"] = res
    _LAST["meta"] = meta
    return _postprocess(res.results, meta)


# revision 32
# speedup vs baseline: 1.7341x; 1.7341x over previous
"""AntiSymmetricConv (GNN message passing) on 8 TRN2 NeuronCores — v2.

Strategy (dst-sharded pull mode, dma_gather based):
  - Host: degree-sort dst nodes, assign 128-node tiles round-robin to the 8
    cores (load balance + one SPMD graph).  Node (core c, tile t, slot p)
    owns y-row R = c*12800 + p*100 + t of the AllGathered y_full.  y_full is
    bf16 and split into 4 windows of 25600 rows (= 2 shards) so that
    window-relative row indices fit dma_gather's int16 index format.
  - Per (dst tile i, src window w), in-edges are packed into chunks of 128;
    the chunk schedule K(i,w) = max over cores (SPMD).  Each edge also
    carries its dst slot (for on-device one-hot scatter matrices).
  - Device, per iteration:
      phase A per tile: transpose x (PE), [xw|xa] = x @ [W_phi.T|A.T] (bf16
        matmul), y = xw*dinv_y -> y_sb (bf16), xa += bias -> xa_sb.
      One flat DMA y_sb -> y_bounce; AllGather -> y_full [102400,128] bf16.
      phase C per group of tiles (packed to <=GCH chunks): one dma_gather
        per (group, window) pulls all chunks' src rows (128 rows/chunk, one
        SWDGE instruction for thousands of descriptors); per tile: one
        batched DVE is_equal generates all one-hot scatter matrices S;
        K matmuls accumulate S^T @ rows into PSUM + identity matmul adds
        the self-loop; fused epilogue h = tanh(agg*dinv + xa), x += 0.1*h
        (x kept in DRAM).
  - Output: read back x shards, invert the permutation on host.
"""

import os

import numpy as np
import ml_dtypes

import concourse.bacc as bacc
import concourse.bass as bass
import concourse.mybir as mybir
import concourse.tile as tile
from concourse.bass_utils import run_bass_kernel_spmd
from concourse.masks import make_identity

GAMMA = 0.1
EPSILON = 0.1
NUM_ITERS = 4
P = 128   # partitions / tile size
C = 8     # cores
D = 128   # feature dim
NT = 100  # tiles per core
NPC = NT * P          # rows per core (12800)
NWIN = 4              # gather windows (2 shards each)
WROWS = 2 * NPC       # rows per window (25600)
ZERO_T = 98           # all-pad tile index on every core
GCH = 112             # max chunks per phase-C group (SBUF budget)
SLAB = 64             # max chunks per dma_gather (8192 idxs, multi-packet)
USE_FP8 = True        # fp8e4 DoubleRow matmuls: 2 chunks per PE instruction

FP = mybir.dt.float32
BF = mybir.dt.bfloat16
F8 = mybir.dt.float8e4
I16 = mybir.dt.int16

BF_NP = ml_dtypes.bfloat16


def _pack_idxs(idx_flat: np.ndarray) -> np.ndarray:
    """[n] -> [128, n//16] int16: idx j at (partition j%16, col j//16),
    replicated x8 across partition groups of 16."""
    n = idx_flat.shape[0]
    assert n % 16 == 0
    t = idx_flat.reshape(n // 16, 16).T.astype(np.int16)
    return np.tile(t, (8, 1))


# ----------------------------------------------------------------- host prep
def _preprocess(x, edge_index, W, W_phi, bias):
    N = x.shape[0]
    src = edge_index[0].astype(np.int64)
    dst = edge_index[1].astype(np.int64)

    deg = np.bincount(dst, minlength=N).astype(np.float64) + 1.0
    dinv = (1.0 / np.sqrt(deg)).astype(np.float32)

    # global degree-descending order; rank r -> (g=r//P, slot=r%P),
    # core = g%C, tile = g//C
    order = np.argsort(-deg, kind="stable")
    rank = np.empty(N, dtype=np.int64)
    rank[order] = np.arange(N)

    g_of = rank // P
    core_of = (g_of % C).astype(np.int64)
    tile_of = (g_of // C).astype(np.int64)
    slot_of = (rank % P).astype(np.int64)
    # tiles 98,99 are all-pad on every core for N=100000 (800 global tiles)
    assert tile_of.max() <= ZERO_T - 1, tile_of.max()

    # y_full row (global) and window-relative index
    yrow = core_of * NPC + slot_of * NT + tile_of
    win_of = core_of // 2
    widx = yrow - win_of * WROWS  # in [0, WROWS)

    # ---- per-core node data (x upload: tile-major rows r = t*128 + p)
    node_at = np.full((C, NT, P), -1, dtype=np.int64)
    node_at[core_of, tile_of, slot_of] = np.arange(N)
    valid = node_at >= 0
    nid = np.where(valid, node_at, 0)
    x_mem = x[nid.reshape(C, -1)].astype(np.float32)   # [C, NT*P, D]
    x_mem[~valid.reshape(C, -1)] = 0.0

    dv = dinv[nid]                                     # [C, NT, P]
    dinv_sb = np.where(valid, dv, 1.0).astype(np.float32)
    dinv_y_sb = np.where(valid, dv, 0.0).astype(np.float32)
    dinv_sb = np.ascontiguousarray(dinv_sb.transpose(0, 2, 1))      # [C,P,NT]
    dinv_y_sb = np.ascontiguousarray(dinv_y_sb.transpose(0, 2, 1))

    # ---- edge schedule
    e_core = core_of[dst]
    e_tile = tile_of[dst]
    e_slot = slot_of[dst]
    e_win = win_of[src]
    e_widx = widx[src]

    key = (e_core * NT + e_tile) * NWIN + e_win
    cnt = np.bincount(key, minlength=C * NT * NWIN).reshape(C, NT, NWIN)
    K_need = -(-cnt // P)
    K_sched = K_need.max(axis=0)               # [NT, NWIN] shared schedule
    CHT = int(K_sched.sum())
    Ktot_i = K_sched.sum(axis=1)               # [NT]
    KMAX = int(Ktot_i.max())

    # order edges by (core, tile, window), stable; position within group
    eorder = np.argsort(key, kind="stable")
    key_s = key[eorder]
    first = np.searchsorted(key_s, key_s)
    j_in_grp = np.arange(len(key_s)) - first

    # tile-major chunk id space: chunk (i, w, k) -> off_iw[i*NWIN+w] + k
    off_iw = np.zeros(NT * NWIN + 1, dtype=np.int64)
    off_iw[1:] = np.cumsum(K_sched.reshape(-1))
    e_i = (key_s // NWIN) % NT
    e_w = key_s % NWIN
    e_c = key_s // (NT * NWIN)
    chunk_of = off_iw[e_i * NWIN + e_w] + j_in_grp // P
    pos_of = j_in_grp % P

    widx_arr = np.full((C, P, CHT), ZERO_T, dtype=np.int32)  # zero row = 98
    sdst_arr = np.zeros((C, P, CHT), dtype=np.float32)
    widx_arr[e_c, pos_of, chunk_of] = e_widx[eorder].astype(np.int32)
    sdst_arr[e_c, pos_of, chunk_of] = e_slot[eorder].astype(np.float32)
    assert widx_arr.max() < WROWS

    # ---- variable groups: pack tiles while sum(Ktot) <= GCH
    groups = []  # list of tile-lists
    cur, cur_k = [], 0
    for i in range(NT):
        k = int(Ktot_i[i])
        if cur and cur_k + k > GCH:
            groups.append(cur)
            cur, cur_k = [], 0
        cur.append(i)
        cur_k += k
    if cur:
        groups.append(cur)
    assert max(int(Ktot_i[i]) for i in range(NT)) <= GCH

    # gather order: for group, for w, for tile in group, chunks k
    gorder = np.empty(CHT, dtype=np.int64)   # tile-major chunk -> gather pos
    gw_chunkoff = []                         # per group: [NWIN+1] offsets
    pos = 0
    for tiles in groups:
        offs = [pos]
        for w in range(NWIN):
            for i in tiles:
                k = int(K_sched[i, w])
                cbase = int(off_iw[i * NWIN + w])
                for kk in range(k):
                    gorder[cbase + kk] = pos
                    pos += 1
            offs.append(pos)
        gw_chunkoff.append(offs)
    assert pos == CHT
    inv_gorder = np.empty(CHT, dtype=np.int64)
    inv_gorder[gorder] = np.arange(CHT)
    widx_g = widx_arr[:, :, inv_gorder]      # chunks in gather order

    idx_packed = np.empty((C, P, 8 * CHT), dtype=np.int16)
    for c in range(C):
        flat = widx_g[c].T.reshape(-1)       # j = chunk*128 + p
        idx_packed[c] = _pack_idxs(flat)

    # matmul rhs [128, 256] bf16 = [W_phi.T | A.T]
    A = W - W.T - GAMMA * np.eye(D, dtype=np.float32)
    rhs = np.concatenate([W_phi.T, A.T], axis=1).astype(BF_NP)
    bias_b = np.tile(bias[None, :], (P, 1)).astype(BF_NP)

    in_maps = []
    for c in range(C):
        in_maps.append(
            {
                "x_in": np.ascontiguousarray(x_mem[c]),
                "dinv": np.ascontiguousarray(dinv_sb[c]),
                "dinv_y": np.ascontiguousarray(dinv_y_sb[c]),
                "idx_pk": np.ascontiguousarray(idx_packed[c]),
                "sdst": np.ascontiguousarray(sdst_arr[c].astype(BF_NP)),
                "rhs": rhs,
                "bias_b": bias_b,
            }
        )
    meta = dict(
        K_sched=K_sched, CHT=CHT, KMAX=KMAX, groups=groups,
        gw_chunkoff=gw_chunkoff, off_iw=off_iw, gorder=gorder,
        node_at=node_at, valid=valid, N=N,
    )
    return in_maps, meta


def _postprocess(results, meta):
    N = meta["N"]
    node_at, valid = meta["node_at"], meta["valid"]
    out = np.empty((N, D), dtype=np.float32)
    for c in range(C):
        xc = results[c]["x_out"].reshape(NT, P, D)
        v = valid[c]
        out[node_at[c][v]] = xc[v]
    return out


# ------------------------------------------------------------- device graph
def _build_graph(meta, n_iters=NUM_ITERS, debug_mode=None):
    K_sched = meta["K_sched"]
    CHT = int(meta["CHT"])
    KMAX = max(int(meta["KMAX"]), 1)
    groups = meta["groups"]
    gw_chunkoff = meta["gw_chunkoff"]
    off_iw = meta["off_iw"]
    gorder = meta["gorder"]

    nc = bacc.Bacc("TRN2", target_bir_lowering=False, debug=False,
                   num_devices=C)
    x_in = nc.declare_dram_parameter("x_in", [NPC, D], FP, isOutput=False)
    dinv_in = nc.declare_dram_parameter("dinv", [P, NT], FP, isOutput=False)
    dinvy_in = nc.declare_dram_parameter("dinv_y", [P, NT], FP,
                                         isOutput=False)
    idx_in = nc.declare_dram_parameter("idx_pk", [P, 8 * CHT], I16,
                                       isOutput=False)
    sdst_in = nc.declare_dram_parameter("sdst", [P, CHT], BF, isOutput=False)
    rhs_in = nc.declare_dram_parameter("rhs", [P, 2 * D], BF, isOutput=False)
    bias_in = nc.declare_dram_parameter("bias_b", [P, D], BF, isOutput=False)
    x_out = nc.declare_dram_parameter("x_out", [NPC, D], FP, isOutput=True)

    x_cur = nc.dram_tensor("x_cur", [NPC, D], FP)
    y_bounce = nc.dram_tensor("y_bounce", [NPC, D], BF)
    y_full = nc.dram_tensor("y_full", [C * NPC, D], BF, addr_space="Shared")

    with tile.TileContext(nc) as tc:
        with (
            tc.tile_pool(name="stat", bufs=1) as stat,
            tc.tile_pool(name="xio", bufs=3) as xio,
            tc.tile_pool(name="xt", bufs=2) as xtp,
            tc.tile_pool(name="gat", bufs=2) as gat,
            tc.tile_pool(name="g8p", bufs=2) as g8p,
            tc.tile_pool(name="idxp", bufs=2) as idxp,
            tc.tile_pool(name="sgen", bufs=2) as sgen,
            tc.tile_pool(name="ps", bufs=2, space="PSUM") as psp,
            tc.tile_pool(name="psagg", bufs=2, space="PSUM") as psagg,
        ):
            # ---- static data
            ident_f = stat.tile([P, P], FP)
            make_identity(nc, ident_f[:])
            ident_b = stat.tile([P, P], BF)
            nc.vector.tensor_copy(out=ident_b[:], in_=ident_f[:])
            iota_rep = stat.tile([P, KMAX * D], BF)
            nc.gpsimd.iota(
                iota_rep[:], pattern=[[0, KMAX], [1, D]], base=0,
                channel_multiplier=0, allow_small_or_imprecise_dtypes=True,
            )
            rhs_sb = stat.tile([P, 2 * D], BF)
            nc.sync.dma_start(rhs_sb[:], rhs_in[:])
            bias_sb = stat.tile([P, D], BF)
            nc.sync.dma_start(bias_sb[:], bias_in[:])
            dinv_sb = stat.tile([P, NT], FP)
            nc.sync.dma_start(dinv_sb[:], dinv_in[:])
            dinvy_sb = stat.tile([P, NT], FP)
            nc.sync.dma_start(dinvy_sb[:], dinvy_in[:])
            sdst_sb = stat.tile([P, CHT], BF)
            nc.sync.dma_start(sdst_sb[:], sdst_in[:])
            y_sb = stat.tile([P, NT * D], BF)
            xa_sb = stat.tile([P, NT * D], BF)



            def phase_a(it):
                for i in range(NT):
                    x_t = xio.tile([P, D], FP, tag="xa_in")
                    src_t = x_in if it == 0 else x_cur
                    nc.sync.dma_start(x_t[:], src_t[i * P:(i + 1) * P, :])
                    ps_t = psp.tile([P, D], FP, tag="ps_t", space="PSUM")
                    nc.tensor.transpose(out=ps_t[:], in_=x_t[:],
                                        identity=ident_f[:])
                    xT = xtp.tile([P, D], BF, tag="xT")
                    nc.vector.tensor_copy(out=xT[:], in_=ps_t[:])
                    ps_a = psp.tile([P, 2 * D], FP, tag="ps_a", space="PSUM")
                    nc.tensor.matmul(out=ps_a[:], lhsT=xT[:], rhs=rhs_sb[:],
                                     start=True, stop=True)
                    nc.vector.tensor_scalar(
                        out=y_sb[:, i * D:(i + 1) * D], in0=ps_a[:, 0:D],
                        scalar1=dinvy_sb[:, i:i + 1], scalar2=None,
                        op0=mybir.AluOpType.mult)
                    nc.vector.tensor_tensor(
                        out=xa_sb[:, i * D:(i + 1) * D], in0=ps_a[:, D:2 * D],
                        in1=bias_sb[:], op=mybir.AluOpType.add)
                    # fold the self-loop term y*dinv into xa (replaces the
                    # per-tile identity matmul in phase C)
                    nc.vector.scalar_tensor_tensor(
                        out=xa_sb[:, i * D:(i + 1) * D],
                        in0=y_sb[:, i * D:(i + 1) * D],
                        scalar=dinv_sb[:, i:i + 1],
                        in1=xa_sb[:, i * D:(i + 1) * D],
                        op0=mybir.AluOpType.mult, op1=mybir.AluOpType.add)
                # flat y copy: y_sb [p, (t d)] -> y_bounce rows p*NT+t
                nc.sync.dma_start(
                    out=y_bounce[:].rearrange("(p t) d -> p (t d)", p=P),
                    in_=y_sb[:])
                if debug_mode == "no_collective":
                    nc.sync.dma_start(out=y_full[0:NPC, :], in_=y_bounce[:])
                else:
                    nc.gpsimd.collective_compute(
                        "AllGather",
                        mybir.AluOpType.bypass,
                        replica_groups=[list(range(C))],
                        ins=[y_bounce[:].opt()],
                        outs=[y_full[:].opt()],
                    )

            def phase_c(it):
                for gi, tiles in enumerate(groups):
                    offs = gw_chunkoff[gi]
                    gbase, gend = int(offs[0]), int(offs[NWIN])
                    nchunks = gend - gbase
                    if nchunks == 0:
                        continue
                    if debug_mode not in ("no_gather", "abl_min"):
                        gbuf = gat.tile([P, GCH, D], BF, tag="gb")
                        idx_t = idxp.tile([P, 8 * GCH], I16, tag="idx")
                        nc.sync.dma_start(
                            idx_t[:, 0:8 * nchunks],
                            idx_in[:, 8 * gbase:8 * gend])
                    for w in range(NWIN):
                        c0, c1 = int(offs[w]), int(offs[w + 1])
                        if c1 == c0 or debug_mode in ("no_gather", "abl_min"):
                            continue
                        # HW caps: 1024 idxs single-packet, 8192 multi-packet
                        for s0 in range(c0, c1, SLAB):
                            s1 = min(s0 + SLAB, c1)
                            ns = s1 - s0
                            nc.gpsimd.dma_gather(
                                out_ap=gbuf[:, s0 - gbase:s1 - gbase, :],
                                in_ap=y_full[w * WROWS:(w + 1) * WROWS, :],
                                idxs_ap=idx_t[:, 8 * (s0 - gbase):
                                              8 * (s1 - gbase)],
                                num_idxs=ns * P,
                                num_idxs_reg=ns * P,
                                elem_size=D,
                                single_packet=(ns * P <= 1024),
                            )
                    if USE_FP8 and debug_mode not in ("no_gather", "abl_min"):
                        g8 = g8p.tile([P, GCH, D], F8, tag="g8")
                        nc.vector.tensor_copy(
                            out=g8[:, 0:nchunks, :],
                            in_=gbuf[:, 0:nchunks, :])
                        mm_src = g8
                        s_dt = F8
                    else:
                        mm_src = None
                        s_dt = BF
                    for i in tiles:
                        kt = int(K_sched[i].sum())
                        # kt==0 (all-pad tiles): agg is empty and xa already
                        # holds t4 = xa + y*dinv; nothing to do.
                        if kt == 0 or debug_mode in ("no_gather", "abl_min"):
                            continue
                        ps_g = psagg.tile([P, D], FP, tag="agg",
                                          space="PSUM")
                        if True:
                            src_buf = mm_src if USE_FP8 else gbuf
                            s_t = sgen.tile([P, KMAX, D], s_dt, tag="s_t")
                            s_off = int(off_iw[i * NWIN])
                            nc.vector.tensor_tensor(
                                out=s_t[:, 0:kt, :],
                                in0=iota_rep[:, 0:kt * D].rearrange(
                                    "p (k d) -> p k d", k=kt),
                                in1=sdst_sb[:, s_off:s_off + kt]
                                .broadcast_to([P, kt, D]),
                                op=mybir.AluOpType.is_equal)
                            # gbuf columns of this tile's chunks, w-major
                            # (strictly increasing, matching s_t column m)
                            cols = []
                            for w in range(NWIN):
                                kw = int(K_sched[i, w])
                                if kw == 0:
                                    continue
                                cc = int(gorder[off_iw[i * NWIN + w]]) - gbase
                                cols.extend(range(cc, cc + kw))
                            m = 0
                            while m < kt:
                                if USE_FP8 and m + 1 < kt:
                                    c1, c2 = cols[m], cols[m + 1]
                                    nc.tensor.matmul(
                                        out=ps_g[:],
                                        lhsT=s_t[:, m:m + 2, :],
                                        rhs=src_buf[:, c1:c2 + 1:c2 - c1, :],
                                        start=(m == 0), stop=(m + 2 >= kt),
                                        perf_mode=mybir.MatmulPerfMode
                                        .DoubleRow)
                                    m += 2
                                else:
                                    nc.tensor.matmul(
                                        out=ps_g[:],
                                        lhsT=s_t[:, m, :],
                                        rhs=src_buf[:, cols[m], :],
                                        start=(m == 0), stop=(m + 1 >= kt))
                                    m += 1
                        # t4 = agg*dinv + xa, overwriting the xa slice
                        nc.vector.scalar_tensor_tensor(
                            out=xa_sb[:, i * D:(i + 1) * D], in0=ps_g[:],
                            scalar=dinv_sb[:, i:i + 1],
                            in1=xa_sb[:, i * D:(i + 1) * D],
                            op0=mybir.AluOpType.mult,
                            op1=mybir.AluOpType.add)
                # batched epilogue: h = tanh(t4) in place, then quartered
                # x update x += eps*h (x stays in DRAM, staged per quarter)
                nc.scalar.activation(
                    out=xa_sb[:], in_=xa_sb[:],
                    func=mybir.ActivationFunctionType.Tanh)
                QT = NT // 4
                for q in range(4):
                    r0, r1 = q * QT * P, (q + 1) * QT * P
                    xq = xio.tile([P, QT * D], FP, tag="xq", bufs=2)
                    src_t = x_in if it == 0 else x_cur
                    nc.sync.dma_start(
                        xq[:].rearrange("p (t d) -> p t d", t=QT),
                        src_t[r0:r1, :].rearrange("(t p) d -> p t d", p=P))
                    nc.vector.scalar_tensor_tensor(
                        out=xq[:], in0=xa_sb[:, q * QT * D:(q + 1) * QT * D],
                        scalar=EPSILON, in1=xq[:],
                        op0=mybir.AluOpType.mult, op1=mybir.AluOpType.add)
                    dst_t = x_out if it == n_iters - 1 else x_cur
                    nc.sync.dma_start(
                        dst_t[r0:r1, :].rearrange("(t p) d -> p t d", p=P),
                        xq[:].rearrange("p (t d) -> p t d", t=QT))

            if n_iters == 0:
                nc.sync.dma_start(out=x_out[:], in_=x_in[:])
                # Fixed per-execution overhead parity with the full graph:
                # the SWDGE gather ucode library load, collective init, and
                # act table load are per-NEFF-execution constants, so the
                # overhead-only graph must pay them too for t_full - t_zero
                # to isolate the marginal compute.
                zscr = nc.dram_tensor("zscr", [P, D], FP)
                zidx = stat.tile([P, 8], I16)
                nc.vector.memset(zidx[:], 0)
                zg = stat.tile([P, 1, D], BF)
                nc.gpsimd.dma_gather(
                    out_ap=zg[:], in_ap=y_full[0:WROWS, :],
                    idxs_ap=zidx[:], num_idxs=P, num_idxs_reg=P,
                    elem_size=D)
                zt = stat.tile([P, D], FP)
                nc.scalar.activation(
                    out=zt[:], in_=zg[:, 0, :],
                    func=mybir.ActivationFunctionType.Tanh)
                nc.sync.dma_start(zscr[:], zt[:])
                nc.gpsimd.collective_compute(
                    "AllGather",
                    mybir.AluOpType.bypass,
                    replica_groups=[list(range(C))],
                    ins=[y_bounce[:].opt()],
                    outs=[y_full[:].opt()],
                )
            for it in range(n_iters):
                phase_a(it)
                phase_c(it)
    nc.compile()
    return nc


# ------------------------------------------------------------------- driver
_LAST = {}


def kernel(x, edge_index, W, W_phi, bias):
    x = np.asarray(x, dtype=np.float32)
    edge_index = np.asarray(edge_index, dtype=np.int32)
    W = np.asarray(W, dtype=np.float32)
    W_phi = np.asarray(W_phi, dtype=np.float32)
    bias = np.asarray(bias, dtype=np.float32)

    in_maps, meta = _preprocess(x, edge_index, W, W_phi, bias)
    nc = _build_graph(meta)
    trace = os.environ.get("BASS_PROFILE", "0") == "1"
    res = run_bass_kernel_spmd(
        nc, in_maps, core_ids=list(range(C)), trace=trace
    )
    _LAST["res"] = res
    _LAST["meta"] = meta
    return _postprocess(res.results, meta)
